# revision 1
# baseline (speedup 1.0000x reference)
"""Cluster-loss (two-view) Trainium2 kernel.

Math:
    f1n = feat1 / ||feat1||_row ;  f2n = feat2 / ||feat2||_row
    s1 = segsum(f1n, label) ; s2 = segsum(f2n, label) ; counts = bincount(label)
    c1 - c2 = (s1 - s2) / max(counts,1)  -> loss = sum(relu(||c1-c2||^2 - margin))

Key identity: s1 - s2 = segsum(f1n - f2n), so the device only computes ONE
segment sum, of h = f1n - f2n.  The segment sum is a one-hot matmul:
    hsegT[d, c] = sum_t h[t, d] * onehot(label[t])[c]
Per 128-token tile:  lhsT (stationary) = u = (f2*r - f1)  [128tok, 128d] fp16
                     rhs  (moving)     = W = (iota==label)*rs1  [128tok, 1024c] fp16
where r = rs2/rs1, rs_i = 1/||f_i||.  Then u.T @ W = -(h)^T-seg contribution
(rs1 scaling folded into W so normalization costs one pass over the data).
PSUM accumulates all 976 tiles in fp32.  Host: all-reduce 8 cores' partials,
remainder tokens (1M - 8*124928 = 576) in numpy, then counts/hinge/sum.

Perf structure:
 - f tiles DMA-cast fp32->fp16 in flight (SWDGE) so every DVE op runs in a
   16-bit perf mode (2x/4x).
 - All W-build operands fp16 to qualify for the 4x tensor_scalar mode.
 - Two-stage software pipeline: batch b's matmul phase is interleaved with
   batch b+1's sum-of-squares phase so the PE never idles >3.4us (HAM stays
   warm).

Sharding: data-parallel over N; core i gets rows [i*124928, (i+1)*124928).
"""

from contextlib import ExitStack

import numpy as np

import concourse.bass as bass
import concourse.mybir as mybir
import concourse.tile as tile
from concourse import bacc
from concourse.bass_utils import run_bass_kernel_spmd

N_CORES = 8
D = 128
C = 1000
CPAD = 1024          # classes padded to 2 PSUM banks / 2x512 matmuls
P = 128              # tokens per tile (matmul K)
TPB = 16             # tiles per DMA batch (1 MiB per view per batch)
N_BATCHES = 61
N_TILES = N_BATCHES * TPB          # 976
SHARD = N_TILES * P                # 124928 tokens per core
MARGIN = 0.1

F32 = mybir.dt.float32
F16 = mybir.dt.float16
AF = mybir.ActivationFunctionType
OP = mybir.AluOpType


def build_nc(n_batches: int = N_BATCHES):
    n_tiles = n_batches * TPB
    shard = n_tiles * P
    # Bacc (not raw Bass): its compile() spills excess sync waits into
    # EventSemaphore instructions — walrus caps most ISA structs at 1 wait.
    nc = bacc.Bacc("TRN2", target_bir_lowering=False, debug=False)

    f1_d = nc.dram_tensor("f1", [shard, D], F32, kind="ExternalInput")
    f2_d = nc.dram_tensor("f2", [shard, D], F32, kind="ExternalInput")
    lab_d = nc.dram_tensor("lab", [P, n_tiles], F32, kind="ExternalInput")
    iota_d = nc.dram_tensor("iota", [P, CPAD], F16, kind="ExternalInput")
    out_d = nc.dram_tensor("hseg", [D, CPAD], F32, kind="ExternalOutput")

    f1r = f1_d.ap().rearrange("(b t p) d -> b p t d", t=TPB, p=P)
    f2r = f2_d.ap().rearrange("(b t p) d -> b p t d", t=TPB, p=P)

    with tile.TileContext(nc) as tc, ExitStack() as ctx:
        const = ctx.enter_context(tc.tile_pool(name="const", bufs=1))
        fpool = ctx.enter_context(tc.tile_pool(name="fpool", bufs=3))
        sqpool = ctx.enter_context(tc.tile_pool(name="sqpool", bufs=2))
        # dead-store `out` of accum Squares; enough bufs that the accum
        # instruction (1 sync-wait slot) never picks up a WAR wait.
        scratch = ctx.enter_context(tc.tile_pool(name="scratch", bufs=34))
        spool = ctx.enter_context(tc.tile_pool(name="spool", bufs=n_batches))
        upool = ctx.enter_context(tc.tile_pool(name="upool", bufs=6))
        wpool = ctx.enter_context(tc.tile_pool(name="wpool", bufs=6))
        ppool = ctx.enter_context(tc.tile_pool(name="ppool", bufs=1, space="PSUM"))

        iota_sb = const.tile([P, CPAD], F16)
        nc.sync.dma_start(iota_sb[:], iota_d[:])
        lab_sb = const.tile([P, n_tiles], F32)
        nc.sync.dma_start(lab_sb[:], lab_d[:])

        psum = ppool.tile([D, CPAD], F32)

        def emit_load(b):
            f1t = fpool.tile([P, TPB, D], F32, name="f1t")
            nc.sync.dma_start(f1t[:], f1r[b])
            f2t = fpool.tile([P, TPB, D], F32, name="f2t")
            nc.sync.dma_start(f2t[:], f2r[b])
            return f1t, f2t

        def emit_sumsq(st, step):
            """Batched per view: one ACT Square over the batch (fp16 out) and
            one DVE reduce per 128-token group; 4 steps interleave with the
            previous batch's matmul phase."""
            if step == 0:
                sq1 = sqpool.tile([P, TPB * D], F16, name="sq1")
                nc.scalar.activation(sq1[:], st["f1t"][:].rearrange("p t d -> p (t d)"),
                                     AF.Square)
                st["sq1"] = sq1
            elif step == 1:
                nc.vector.tensor_reduce(
                    st["ss1"][:], st["sq1"][:].rearrange("p (t d) -> p t d", d=D),
                    axis=mybir.AxisListType.X, op=OP.add,
                )
            elif step == 2:
                sq2 = sqpool.tile([P, TPB * D], F16, name="sq2")
                nc.scalar.activation(sq2[:], st["f2t"][:].rearrange("p t d -> p (t d)"),
                                     AF.Square)
                st["sq2"] = sq2
            elif step == 3:
                nc.vector.tensor_reduce(
                    st["ss2"][:], st["sq2"][:].rearrange("p (t d) -> p t d", d=D),
                    axis=mybir.AxisListType.X, op=OP.add,
                )

        def emit_stats(st):
            """Batch-level: rs1 and r = rs2/rs1 from ss1/ss2."""
            ss1, ss2 = st["ss1"], st["ss2"]
            sqr1 = spool.tile([P, TPB], F32, name="sqr1")
            nc.scalar.activation(sqr1[:], ss1[:], AF.Sqrt)   # ||f1||
            sqr2 = spool.tile([P, TPB], F32, name="sqr2")
            nc.scalar.activation(sqr2[:], ss2[:], AF.Sqrt)   # ||f2||
            inv1 = spool.tile([P, TPB], F32, name="inv1")
            nc.vector.reciprocal(inv1[:], sqr1[:])           # rs1
            inv2 = spool.tile([P, TPB], F32, name="inv2")
            nc.vector.reciprocal(inv2[:], sqr2[:])           # rs2
            rh = spool.tile([P, TPB], F32, name="rh")
            nc.vector.tensor_tensor(rh[:], sqr1[:], inv2[:], OP.mult)  # rs2/rs1
            st["inv1"], st["rh"] = inv1, rh

        def emit_mm(st, t):
            """Per-tile weights u (GPSIMD), one-hot W (DVE), two matmuls."""
            f1t, f2t = st["f1t"], st["f2t"]
            ti = st["b"] * TPB + t
            # u = f2*r - f1   (= -h/rs1): scale on ACT, subtract on GPSIMD —
            # keeps the DVE free for the one-hot build.
            t2 = upool.tile([P, D], F32, name="t2")
            nc.scalar.activation(
                t2[:], f2t[:, t, :], AF.Copy, bias=0.0,
                scale=st["rh"][:, t : t + 1],
            )
            u = upool.tile([P, D], F16, name="u")
            nc.gpsimd.tensor_tensor(u[:], t2[:], f1t[:, t, :], OP.subtract)
            w = wpool.tile([P, CPAD], F16, name="w")
            nc.vector.tensor_scalar(
                out=w[:], in0=iota_sb[:],
                scalar1=lab_sb[:, ti : ti + 1],
                scalar2=st["inv1"][:, t : t + 1],
                op0=OP.is_equal, op1=OP.mult,
            )
            first = ti == 0
            last = ti == n_tiles - 1
            nc.tensor.matmul(
                psum[:, 0:512], u[:], w[:, 0:512], start=first, stop=last
            )
            nc.tensor.matmul(
                psum[:, 512:CPAD], u[:], w[:, 512:CPAD], start=first, stop=last
            )

        # two-stage software pipeline over batches
        prev = None
        for b in range(n_batches + 1):
            cur = None
            if b < n_batches:
                f1t, f2t = emit_load(b)
                cur = {
                    "b": b, "f1t": f1t, "f2t": f2t,
                    "ss1": spool.tile([P, TPB], F32, name="ss1"),
                    "ss2": spool.tile([P, TPB], F32, name="ss2"),
                }
            for t in range(TPB):
                if prev is not None:
                    emit_mm(prev, t)
                if cur is not None and t % 4 == 1:
                    emit_sumsq(cur, t // 4)
            if cur is not None:
                emit_stats(cur)
            prev = cur

        outsb = const.tile([D, CPAD], F32)
        nc.scalar.copy(outsb[:], psum[:])
        nc.sync.dma_start(out_d[:], outsb[:])

    nc.compile()
    return nc


_NC_CACHE = {}


def _get_nc(n_batches: int = N_BATCHES):
    if n_batches not in _NC_CACHE:
        _NC_CACHE[n_batches] = build_nc(n_batches)
    return _NC_CACHE[n_batches]


def make_in_maps(feat1, feat2, label1, n_batches: int = N_BATCHES):
    shard = n_batches * TPB * P
    iota = np.ascontiguousarray(
        np.broadcast_to(np.arange(CPAD, dtype=np.float16), (P, CPAD))
    )
    in_maps = []
    for c in range(N_CORES):
        lo = c * shard
        lab = (
            label1[lo : lo + shard]
            .astype(np.float32)
            .reshape(n_batches * TPB, P)
            .T.copy()
        )
        in_maps.append(
            {
                "f1": feat1[lo : lo + shard],
                "f2": feat2[lo : lo + shard],
                "lab": lab,
                "iota": iota,
            }
        )
    return in_maps


def finish_host(hsegT_list, feat1, feat2, label1, used: int):
    """Combine per-core partials + host remainder -> scalar loss (float32)."""
    # device psum[d, c] = sum_t (f2*rs2 - f1*rs1)[t, d] * onehot[t, c] = -(s1-s2)^T
    hseg = np.zeros((D, C), dtype=np.float64)
    for h in hsegT_list:
        hseg += h[:, :C].astype(np.float64)
    rem1 = feat1[used:].astype(np.float64)
    rem2 = feat2[used:].astype(np.float64)
    reml = label1[used:]
    if rem1.shape[0]:
        n1 = np.sqrt((rem1 * rem1).sum(1, keepdims=True))
        n2 = np.sqrt((rem2 * rem2).sum(1, keepdims=True))
        hrem = rem1 / n1 - rem2 / n2  # [r, D]
        np.add.at(hseg.T, reml, -hrem)  # device sign convention: -(h)
    counts = np.bincount(label1, minlength=C).astype(np.float64)
    denom = np.maximum(counts, 1.0)
    cdiff = hseg / denom[None, :]
    per_class = (cdiff * cdiff).sum(0)
    hinge = np.maximum(per_class - MARGIN, 0.0)
    hinge = np.where(counts > 0, hinge, 0.0)
    return np.array(hinge.sum(), dtype=np.float32)


def kernel(feat1, feat2, label1, trace: bool = False):
    feat1 = np.ascontiguousarray(np.asarray(feat1, dtype=np.float32))
    feat2 = np.ascontiguousarray(np.asarray(feat2, dtype=np.float32))
    label1 = np.asarray(label1).astype(np.int64)

    in_maps = make_in_maps(feat1, feat2, label1)
    nc = _get_nc()
    res = run_bass_kernel_spmd(
        nc, in_maps, core_ids=list(range(N_CORES)), trace=trace
    )
    hsegs = [res.results[i]["hseg"] for i in range(N_CORES)]
    out = finish_host(hsegs, feat1, feat2, label1, used=N_CORES * SHARD)
    if trace:
        return out, res
    return out



# revision 2
# speedup vs baseline: 1.4146x; 1.4146x over previous
"""Cluster-loss (two-view) Trainium2 kernel — label-sorted windowed segsum.

Math:
    f1n = feat1 / ||feat1||_row ;  f2n = feat2 / ||feat2||_row
    hseg = segsum(f1n - f2n, label)  (= s1 - s2)
    loss = sum_over_present_classes relu(||hseg_c / max(cnt_c,1)||^2 - margin)

Device strategy (per core):
  Host sorts all tokens by label and assigns each core a contiguous range of
  whole classes with ~N/8 tokens (classes are disjoint across cores -> no
  all-reduce; per-core partial IS the final segment sum for its classes).
  Because tokens arrive class-sorted, each 2048-token batch only touches a
  ~3-class span whose position follows a fixed schedule (1000 tokens/class);
  the one-hot matmul thus needs only a WIN=24-column window instead of 1024:
      psum[d, BASE(b):BASE(b)+24] += f_tile[tok,d]^T @ W[tok,24]
  W[tok, j] = (iota[j] == lab_rel[tok]) * rs[tok],  lab_rel = local_class -
  BASE(b) (host-computed; tokens that would fall outside the window - none in
  practice - are routed to an exact host fp64 fallback path instead).
  feat2 is negated on the host so one PSUM region accumulates s1 - s2.

Per-batch device pipeline (five stages, software-pipelined):
  S0 DMA:  SWDGE loads fp32->fp16 cast in flight (8 KiB contig / partition)
  S1 ACT:  sq = Square(f)  [128, 2048] batched
  S2 DVE:  4 halving adds (fp16 2x mode) + tensor_reduce -> ss[tok]
  S3 ACT:  rs = 1/sqrt(ss)  (Abs_reciprocal_sqrt LUT)
  S4 DVE+PE: per 128-token tile: W build (tensor_scalar is_eq*mult) + matmul

Sharding: class-range partition of the globally label-sorted token order.
"""

from contextlib import ExitStack

import numpy as np

import concourse.bass as bass
import concourse.mybir as mybir
import concourse.tile as tile
from concourse import bacc
from concourse.bass_utils import run_bass_kernel_spmd

N_CORES = 8
D = 128
C = 1000
P = 128                  # tokens per tile (matmul K)
TPB = 16                 # tiles per batch
TOK_B = P * TPB          # 2048 tokens per batch
NB = 62                  # batches per core
NT = NB * TPB            # 992 tiles per core
SHARD = NB * TOK_B       # 126976 token capacity per core
WIN = 24                 # one-hot window columns
GUARD = 8                # window guard below predicted class position
OUTW = 192               # per-core PSUM output width (>= max classes/core)
MARGIN = 0.1
TOK_PER_CLASS = 1000.0   # schedule rate: expected tokens per class

F32 = mybir.dt.float32
F16 = mybir.dt.float16
AF = mybir.ActivationFunctionType
OP = mybir.AluOpType


def win_base(b: int) -> int:
    """Fixed per-batch window base (local class index), even-aligned."""
    pred = int(b * TOK_B / TOK_PER_CLASS) - GUARD
    pred &= ~1
    return max(0, min(OUTW - WIN, pred))


def build_nc(nb: int = NB):
    nt = nb * TPB
    shard = nt * P
    nc = bacc.Bacc("TRN2", target_bir_lowering=False, debug=False)

    f1_d = nc.dram_tensor("f1", [shard, D], F32, kind="ExternalInput")
    f2_d = nc.dram_tensor("f2", [shard, D], F32, kind="ExternalInput")
    lab_d = nc.dram_tensor("lab", [P, nt], F32, kind="ExternalInput")
    iota_d = nc.dram_tensor("iota", [P, WIN], F16, kind="ExternalInput")
    out_d = nc.dram_tensor("hseg", [D, OUTW], F32, kind="ExternalOutput")

    # token s = b*2048 + p*16 + t  ->  partition p reads 16*128 contiguous fp32
    f1r = f1_d.ap().rearrange("(b p t) d -> b p (t d)", p=P, t=TPB)
    f2r = f2_d.ap().rearrange("(b p t) d -> b p (t d)", p=P, t=TPB)

    with tile.TileContext(nc) as tc, ExitStack() as ctx:
        const = ctx.enter_context(tc.tile_pool(name="const", bufs=1))
        fpool = ctx.enter_context(tc.tile_pool(name="fpool", bufs=7))
        sqpool = ctx.enter_context(tc.tile_pool(name="sqpool", bufs=3))
        trpool = ctx.enter_context(tc.tile_pool(name="trpool", bufs=3))
        spool = ctx.enter_context(tc.tile_pool(name="spool", bufs=nb))
        wpool = ctx.enter_context(tc.tile_pool(name="wpool", bufs=10))
        ppool = ctx.enter_context(tc.tile_pool(name="ppool", bufs=1, space="PSUM"))

        iota_sb = const.tile([P, WIN], F16)
        nc.sync.dma_start(iota_sb[:], iota_d[:])
        lab_sb = const.tile([P, nt], F32)
        nc.sync.dma_start(lab_sb[:], lab_d[:])
        zeros = const.tile([P, OUTW], F16)
        nc.vector.memset(zeros[:], 0.0)

        psum = ppool.tile([D, OUTW], F32)
        # zero the whole accumulation window: 0^T @ 0
        nc.tensor.matmul(psum[:], zeros[:, 0:P], zeros[:], start=True, stop=False)

        def emit_load(b, st):
            f1t = fpool.tile([P, TPB * D], F16, name="f1t")
            nc.gpsimd.dma_start(f1t[:], f1r[b])
            f2t = fpool.tile([P, TPB * D], F16, name="f2t")
            nc.gpsimd.dma_start(f2t[:], f2r[b])
            st["f1t"], st["f2t"] = f1t, f2t

        def emit_square(st):
            for v in (1, 2):
                sq = sqpool.tile([P, TPB * D], F16, name=f"sq{v}")
                nc.scalar.activation(sq[:], st[f"f{v}t"][:], AF.Square)
                st[f"sq{v}"] = sq

        def emit_tree(st):
            for v in (1, 2):
                src = st[f"sq{v}"][:].rearrange("p (t d) -> p t d", d=D)
                w = D
                while w > 8:
                    w //= 2
                    dst_t = trpool.tile([P, TPB * w], F16, name=f"l{w}_{v}")
                    dst = dst_t[:].rearrange("p (t d) -> p t d", d=w)
                    nc.vector.tensor_tensor(
                        dst, src[:, :, 0:w], src[:, :, w : 2 * w], OP.add
                    )
                    src = dst
                ss = spool.tile([P, TPB], F32, name=f"ss{v}")
                nc.vector.tensor_reduce(
                    ss[:], src, axis=mybir.AxisListType.X, op=OP.add
                )
                st[f"ss{v}"] = ss

        def emit_rs(st):
            for v in (1, 2):
                rs = spool.tile([P, TPB], F32, name=f"rs{v}")
                nc.scalar.activation(rs[:], st[f"ss{v}"][:], AF.Abs_reciprocal_sqrt)
                st[f"rs{v}"] = rs

        def emit_mm(st):
            b = st["b"]
            base = win_base(b)
            last_b = b == nb - 1
            for t in range(TPB):
                ti = b * TPB + t
                for v in (1, 2):
                    w = wpool.tile([P, WIN], F16, name=f"w{v}")
                    nc.vector.tensor_scalar(
                        out=w[:], in0=iota_sb[:],
                        scalar1=lab_sb[:, ti : ti + 1],
                        scalar2=st[f"rs{v}"][:, t : t + 1],
                        op0=OP.is_equal, op1=OP.mult,
                    )
                    stop = last_b and t == TPB - 1 and v == 2
                    nc.tensor.matmul(
                        psum[:, base : base + WIN],
                        st[f"f{v}t"][:, t * D : (t + 1) * D],
                        w[:],
                        start=False, stop=stop,
                    )

        # five-stage software pipeline over batches
        sts = {}
        for k in range(nb + 5):
            if k < nb:
                sts[k] = {"b": k}
                emit_load(k, sts[k])
            if 0 <= k - 2 < nb:
                emit_square(sts[k - 2])
            if 0 <= k - 3 < nb:
                emit_tree(sts[k - 3])
            if 0 <= k - 4 < nb:
                emit_rs(sts[k - 4])
            if 0 <= k - 5 < nb:
                emit_mm(sts[k - 5])
                del sts[k - 5]

        outsb = const.tile([D, OUTW], F32)
        nc.scalar.copy(outsb[:], psum[:])
        nc.sync.dma_start(out_d[:], outsb[:])

    nc.compile()
    return nc


_NC_CACHE = {}


def _get_nc(nb: int = NB):
    if nb not in _NC_CACHE:
        _NC_CACHE[nb] = build_nc(nb)
    return _NC_CACHE[nb]


def _plan_shards(label: np.ndarray):
    """Class-range partition of the sorted tokens: 8 ranges of whole classes,
    each with <= SHARD tokens. Returns (order, class_bounds, counts)."""
    order = np.argsort(label, kind="stable")
    counts = np.bincount(label, minlength=C)
    cum = np.concatenate([[0], np.cumsum(counts)])  # [C+1]
    n = label.shape[0]
    bounds = [0]
    for i in range(1, N_CORES):
        target = round(i * n / N_CORES)
        c = int(np.searchsorted(cum, target))
        if c > 0 and abs(cum[c - 1] - target) < abs(cum[c] - target):
            c -= 1
        c = max(c, bounds[-1])
        bounds.append(min(c, C))
    bounds.append(C)
    # enforce per-range capacity
    for _ in range(C):
        ok = True
        for i in range(N_CORES):
            if cum[bounds[i + 1]] - cum[bounds[i]] > SHARD:
                if i + 1 < N_CORES:
                    bounds[i + 1] -= 1
                else:
                    bounds[i] += 1  # steal from previous range
                ok = False
        if ok:
            break
    for i in range(N_CORES):
        cnt = cum[bounds[i + 1]] - cum[bounds[i]]
        assert cnt <= SHARD, f"shard {i} over capacity: {cnt}"
        assert bounds[i + 1] - bounds[i] <= OUTW, "class range too wide"
    return order, bounds, counts, cum


def make_in_maps(feat1, feat2, label):
    order, bounds, counts, cum = _plan_shards(label)
    sorted_labels = label[order]
    iota = np.ascontiguousarray(
        np.broadcast_to(np.arange(WIN, dtype=np.float16), (P, WIN))
    )
    base_of_batch = np.array([win_base(b) for b in range(NB)], dtype=np.int64)
    base_per_tok = np.repeat(base_of_batch, TOK_B)  # [SHARD]

    in_maps, spill_idx = [], []
    for i in range(N_CORES):
        s0, s1 = int(cum[bounds[i]]), int(cum[bounds[i + 1]])
        cnt = s1 - s0
        sel = order[s0:s1]
        f1c = np.ones((SHARD, D), dtype=np.float32)
        f1c[:cnt] = feat1[sel]
        f2c = np.ones((SHARD, D), dtype=np.float32)
        np.negative(feat2[sel], out=f2c[:cnt])
        local = sorted_labels[s0:s1] - bounds[i]
        lab_rel = local - base_per_tok[:cnt]
        bad = (lab_rel < 0) | (lab_rel >= WIN)
        if bad.any():
            spill_idx.append(sel[bad])
            lab_rel = lab_rel.copy()
            lab_rel[bad] = -1000
        lab_full = np.full(SHARD, -1000.0, dtype=np.float32)
        lab_full[:cnt] = lab_rel.astype(np.float32)
        lab_dev = np.ascontiguousarray(
            lab_full.reshape(NB, P, TPB).transpose(1, 0, 2).reshape(P, NT)
        )
        in_maps.append({"f1": f1c, "f2": f2c, "lab": lab_dev, "iota": iota})
    spill = np.concatenate(spill_idx) if spill_idx else np.empty(0, dtype=np.int64)
    return in_maps, bounds, counts, spill


def finish_host(outs, bounds, counts, spill, feat1, feat2, label):
    hseg = np.zeros((D, C), dtype=np.float64)
    for i in range(N_CORES):
        k = bounds[i + 1] - bounds[i]
        hseg[:, bounds[i] : bounds[i + 1]] += outs[i][:, :k].astype(np.float64)
    if spill.size:
        r1 = feat1[spill].astype(np.float64)
        r2 = feat2[spill].astype(np.float64)
        h = r1 / np.sqrt((r1 * r1).sum(1, keepdims=True)) - r2 / np.sqrt(
            (r2 * r2).sum(1, keepdims=True)
        )
        np.add.at(hseg.T, label[spill], h)
    denom = np.maximum(counts, 1.0)
    cdiff = hseg / denom[None, :]
    per_class = (cdiff * cdiff).sum(0)
    hinge = np.maximum(per_class - MARGIN, 0.0)
    hinge = np.where(counts > 0, hinge, 0.0)
    return np.array(hinge.sum(), dtype=np.float32)


def kernel(feat1, feat2, label1, trace: bool = False):
    feat1 = np.ascontiguousarray(np.asarray(feat1, dtype=np.float32))
    feat2 = np.ascontiguousarray(np.asarray(feat2, dtype=np.float32))
    label = np.asarray(label1).astype(np.int64)

    in_maps, bounds, counts, spill = make_in_maps(feat1, feat2, label)
    nc = _get_nc()
    res = run_bass_kernel_spmd(
        nc, in_maps, core_ids=list(range(N_CORES)), trace=trace
    )
    outs = [res.results[i]["hseg"] for i in range(N_CORES)]
    out = finish_host(outs, bounds, counts, spill, feat1, feat2, label)
    if trace:
        return out, res
    return out


# revision 7
# speedup vs baseline: 2.4199x; 1.7107x over previous
"""Cluster-loss (two-view) Trainium2 kernel — label-sorted windowed segsum.

Math:
    f1n = feat1 / ||feat1||_row ;  f2n = feat2 / ||feat2||_row
    hseg = segsum(f1n - f2n, label)  (= s1 - s2)
    loss = sum_over_present_classes relu(||hseg_c / max(cnt_c,1)||^2 - margin)

Device strategy (per core):
  Host sorts all tokens by label and assigns each core a contiguous range of
  whole classes with ~N/8 tokens (classes are disjoint across cores -> no
  all-reduce; per-core partial IS the final segment sum for its classes).
  Because tokens arrive class-sorted, each 2048-token batch only touches a
  ~3-class span whose position follows a fixed schedule (1000 tokens/class);
  the one-hot matmul thus needs only a WIN=24-column window instead of 1024:
      psum[d, BASE(b):BASE(b)+24] += f_tile[tok,d]^T @ W[tok,24]
  W[tok, j] = (iota[j] == lab_rel[tok]) * rs[tok],  lab_rel = local_class -
  BASE(b) (host-computed; tokens that would fall outside the window - none in
  practice - are routed to an exact host fp64 fallback path instead).
  feat2 is negated on the host so one PSUM region accumulates s1 - s2.

Per-batch device pipeline (five stages, software-pipelined):
  S0 DMA:  SWDGE loads fp32->fp16 cast in flight (8 KiB contig / partition)
  S1 ACT:  sq = Square(f)  [128, 2048] batched
  S2 DVE:  4 halving adds (fp16 2x mode) + tensor_reduce -> ss[tok]
  S3 ACT:  rs = 1/sqrt(ss)  (Abs_reciprocal_sqrt LUT)
  S4 DVE+PE: per 128-token tile: W build (tensor_scalar is_eq*mult) + matmul

Sharding: class-range partition of the globally label-sorted token order.
"""

from contextlib import ExitStack

import numpy as np

import concourse.bass as bass
import concourse.mybir as mybir
import concourse.tile as tile
from concourse import bacc
from concourse.bass_utils import run_bass_kernel_spmd

N_CORES = 8
D = 128
C = 1000
P = 128                  # tokens per tile (matmul K)
TPB = 16                 # tiles per batch
TOK_B = P * TPB          # 2048 tokens per batch
NB = 62                  # batches per core
NT = NB * TPB            # 992 tiles per core
SHARD = NB * TOK_B       # 126976 token capacity per core
WIN = 24                 # one-hot window columns
GUARD = 8                # window guard below predicted class position
OUTW = 192               # per-core PSUM output width (>= max classes/core)
MARGIN = 0.1
TOK_PER_CLASS = 1000.0   # schedule rate: expected tokens per class

F32 = mybir.dt.float32
F16 = mybir.dt.float16
AF = mybir.ActivationFunctionType
OP = mybir.AluOpType


def win_base(b: int) -> int:
    """Fixed per-batch window base (local class index), even-aligned."""
    pred = int(b * TOK_B / TOK_PER_CLASS) - GUARD
    pred &= ~1
    return max(0, min(OUTW - WIN, pred))


def build_nc(nb: int = NB):
    nt = nb * TPB
    shard = nt * P
    nc = bacc.Bacc("TRN2", target_bir_lowering=False, debug=False)

    f1_d = nc.dram_tensor("f1", [shard, D], F32, kind="ExternalInput")
    f2_d = nc.dram_tensor("f2", [shard, D], F32, kind="ExternalInput")
    lab_d = nc.dram_tensor("lab", [P, nt], F32, kind="ExternalInput")
    iota_d = nc.dram_tensor("iota", [P, TPB * WIN], F16, kind="ExternalInput")
    out_d = nc.dram_tensor("hseg", [D, OUTW], F32, kind="ExternalOutput")

    # token s = b*2048 + p*16 + t  ->  partition p reads 16*128 contiguous fp32
    f1r = f1_d.ap().rearrange("(b p t) d -> b p (t d)", p=P, t=TPB)
    f2r = f2_d.ap().rearrange("(b p t) d -> b p (t d)", p=P, t=TPB)

    with tile.TileContext(nc) as tc, ExitStack() as ctx:
        const = ctx.enter_context(tc.tile_pool(name="const", bufs=1))
        fpool = ctx.enter_context(tc.tile_pool(name="fpool", bufs=7))
        sqpool = ctx.enter_context(tc.tile_pool(name="sqpool", bufs=3))
        trpool = ctx.enter_context(tc.tile_pool(name="trpool", bufs=3))
        spool = ctx.enter_context(tc.tile_pool(name="spool", bufs=nb))
        wpool = ctx.enter_context(tc.tile_pool(name="wpool", bufs=10))
        ppool = ctx.enter_context(tc.tile_pool(name="ppool", bufs=1, space="PSUM"))

        iota_sb = const.tile([P, TPB * WIN], F16)
        nc.sync.dma_start(iota_sb[:], iota_d[:])
        lab_sb = const.tile([P, nt], F32)
        nc.sync.dma_start(lab_sb[:], lab_d[:])
        zeros = const.tile([P, OUTW], F16)
        nc.vector.memset(zeros[:], 0.0)

        psum = ppool.tile([D, OUTW], F32)
        # zero the whole accumulation window: 0^T @ 0
        nc.tensor.matmul(psum[:], zeros[:, 0:P], zeros[:], start=True, stop=False)

        def emit_load(b, st):
            f1t = fpool.tile([P, TPB * D], F16, name="f1t")
            nc.gpsimd.dma_start(f1t[:], f1r[b])
            f2t = fpool.tile([P, TPB * D], F16, name="f2t")
            nc.gpsimd.dma_start(f2t[:], f2r[b])
            st["f1t"], st["f2t"] = f1t, f2t

        def emit_square(st):
            for v in (1, 2):
                sq = sqpool.tile([P, TPB * D], F16, name=f"sq{v}")
                nc.scalar.activation(sq[:], st[f"f{v}t"][:], AF.Square)
                st[f"sq{v}"] = sq

        def emit_tree(st):
            for v in (1, 2):
                src = st[f"sq{v}"][:].rearrange("p (t d) -> p t d", d=D)
                w = D
                while w > 8:
                    w //= 2
                    dst_t = trpool.tile([P, TPB * w], F16, name=f"l{w}_{v}")
                    dst = dst_t[:].rearrange("p (t d) -> p t d", d=w)
                    nc.vector.tensor_tensor(
                        dst, src[:, :, 0:w], src[:, :, w : 2 * w], OP.add
                    )
                    src = dst
                ss = spool.tile([P, TPB], F32, name=f"ss{v}")
                nc.vector.tensor_reduce(
                    ss[:], src, axis=mybir.AxisListType.X, op=OP.add
                )
                st[f"ss{v}"] = ss

        def emit_rs(st):
            for v in (1, 2):
                rs = spool.tile([P, TPB], F32, name=f"rs{v}")
                nc.scalar.activation(rs[:], st[f"ss{v}"][:], AF.Abs_reciprocal_sqrt)
                st[f"rs{v}"] = rs

        def emit_w(st):
            """Whole-batch one-hot build: 3 DVE ops instead of 32.
            eq[p, t, j] = (iota[j] == lab_rel[p, t]);  w_v = eq * rs_v[p, t]."""
            b = st["b"]
            sl = slice(b * TPB, (b + 1) * TPB)
            eq = wpool.tile([P, TPB * WIN], F16, name="eq")
            lab_bc = lab_sb[:, sl].unsqueeze(-1).broadcast_to([P, TPB, WIN])
            nc.vector.tensor_tensor(
                eq[:].rearrange("p (t j) -> p t j", j=WIN),
                iota_sb[:].rearrange("p (t j) -> p t j", j=WIN),
                lab_bc,
                OP.is_equal,
            )
            for v in (1, 2):
                w = wpool.tile([P, TPB * WIN], F16, name=f"w{v}")
                rs_bc = st[f"rs{v}"][:].unsqueeze(-1).broadcast_to([P, TPB, WIN])
                nc.vector.tensor_tensor(
                    w[:].rearrange("p (t j) -> p t j", j=WIN),
                    eq[:].rearrange("p (t j) -> p t j", j=WIN),
                    rs_bc,
                    OP.mult,
                )
                st[f"w{v}"] = w

        def emit_mm(st):
            b = st["b"]
            base = win_base(b)
            last_b = b == nb - 1
            for t in range(TPB):
                for v in (1, 2):
                    stop = last_b and t == TPB - 1 and v == 2
                    nc.tensor.matmul(
                        psum[:, base : base + WIN],
                        st[f"f{v}t"][:, t * D : (t + 1) * D],
                        st[f"w{v}"][:, t * WIN : (t + 1) * WIN],
                        start=False, stop=stop,
                    )

        # five-stage software pipeline over batches
        sts = {}
        for k in range(nb + 5):
            if k < nb:
                sts[k] = {"b": k}
                emit_load(k, sts[k])
            if 0 <= k - 2 < nb:
                emit_square(sts[k - 2])
            if 0 <= k - 3 < nb:
                emit_tree(sts[k - 3])
            if 0 <= k - 4 < nb:
                emit_rs(sts[k - 4])
            if 0 <= k - 5 < nb:
                emit_w(sts[k - 5])
                emit_mm(sts[k - 5])
                del sts[k - 5]

        outsb = const.tile([D, OUTW], F32)
        nc.scalar.copy(outsb[:], psum[:])
        nc.sync.dma_start(out_d[:], outsb[:])

    nc.compile()
    return nc


_NC_CACHE = {}


def _get_nc(nb: int = NB):
    if nb not in _NC_CACHE:
        _NC_CACHE[nb] = build_nc(nb)
    return _NC_CACHE[nb]


def _plan_shards(label: np.ndarray):
    """Class-range partition of the sorted tokens: 8 ranges of whole classes,
    each with <= SHARD tokens. Returns (order, class_bounds, counts)."""
    order = np.argsort(label, kind="stable")
    counts = np.bincount(label, minlength=C)
    cum = np.concatenate([[0], np.cumsum(counts)])  # [C+1]
    n = label.shape[0]
    bounds = [0]
    for i in range(1, N_CORES):
        target = round(i * n / N_CORES)
        c = int(np.searchsorted(cum, target))
        if c > 0 and abs(cum[c - 1] - target) < abs(cum[c] - target):
            c -= 1
        c = max(c, bounds[-1])
        bounds.append(min(c, C))
    bounds.append(C)
    # enforce per-range capacity
    for _ in range(C):
        ok = True
        for i in range(N_CORES):
            if cum[bounds[i + 1]] - cum[bounds[i]] > SHARD:
                if i + 1 < N_CORES:
                    bounds[i + 1] -= 1
                else:
                    bounds[i] += 1  # steal from previous range
                ok = False
        if ok:
            break
    for i in range(N_CORES):
        cnt = cum[bounds[i + 1]] - cum[bounds[i]]
        assert cnt <= SHARD, f"shard {i} over capacity: {cnt}"
        assert bounds[i + 1] - bounds[i] <= OUTW, "class range too wide"
    return order, bounds, counts, cum


def make_in_maps(feat1, feat2, label):
    order, bounds, counts, cum = _plan_shards(label)
    sorted_labels = label[order]
    iota = np.ascontiguousarray(
        np.broadcast_to(
            np.tile(np.arange(WIN, dtype=np.float16), TPB), (P, TPB * WIN)
        )
    )
    base_of_batch = np.array([win_base(b) for b in range(NB)], dtype=np.int64)
    base_per_tok = np.repeat(base_of_batch, TOK_B)  # [SHARD]

    in_maps, spill_idx = [], []
    for i in range(N_CORES):
        s0, s1 = int(cum[bounds[i]]), int(cum[bounds[i + 1]])
        cnt = s1 - s0
        sel = order[s0:s1]
        f1c = np.ones((SHARD, D), dtype=np.float32)
        f1c[:cnt] = feat1[sel]
        f2c = np.ones((SHARD, D), dtype=np.float32)
        np.negative(feat2[sel], out=f2c[:cnt])
        local = sorted_labels[s0:s1] - bounds[i]
        lab_rel = local - base_per_tok[:cnt]
        bad = (lab_rel < 0) | (lab_rel >= WIN)
        if bad.any():
            spill_idx.append(sel[bad])
            lab_rel = lab_rel.copy()
            lab_rel[bad] = -1000
        lab_full = np.full(SHARD, -1000.0, dtype=np.float32)
        lab_full[:cnt] = lab_rel.astype(np.float32)
        lab_dev = np.ascontiguousarray(
            lab_full.reshape(NB, P, TPB).transpose(1, 0, 2).reshape(P, NT)
        )
        in_maps.append({"f1": f1c, "f2": f2c, "lab": lab_dev, "iota": iota})
    spill = np.concatenate(spill_idx) if spill_idx else np.empty(0, dtype=np.int64)
    return in_maps, bounds, counts, spill


def finish_host(outs, bounds, counts, spill, feat1, feat2, label):
    hseg = np.zeros((D, C), dtype=np.float64)
    for i in range(N_CORES):
        k = bounds[i + 1] - bounds[i]
        hseg[:, bounds[i] : bounds[i + 1]] += outs[i][:, :k].astype(np.float64)
    if spill.size:
        r1 = feat1[spill].astype(np.float64)
        r2 = feat2[spill].astype(np.float64)
        h = r1 / np.sqrt((r1 * r1).sum(1, keepdims=True)) - r2 / np.sqrt(
            (r2 * r2).sum(1, keepdims=True)
        )
        np.add.at(hseg.T, label[spill], h)
    denom = np.maximum(counts, 1.0)
    cdiff = hseg / denom[None, :]
    per_class = (cdiff * cdiff).sum(0)
    hinge = np.maximum(per_class - MARGIN, 0.0)
    hinge = np.where(counts > 0, hinge, 0.0)
    return np.array(hinge.sum(), dtype=np.float32)


def kernel(feat1, feat2, label1, trace: bool = False):
    feat1 = np.ascontiguousarray(np.asarray(feat1, dtype=np.float32))
    feat2 = np.ascontiguousarray(np.asarray(feat2, dtype=np.float32))
    label = np.asarray(label1).astype(np.int64)

    in_maps, bounds, counts, spill = make_in_maps(feat1, feat2, label)
    nc = _get_nc()
    res = run_bass_kernel_spmd(
        nc, in_maps, core_ids=list(range(N_CORES)), trace=trace
    )
    outs = [res.results[i]["hseg"] for i in range(N_CORES)]
    out = finish_host(outs, bounds, counts, spill, feat1, feat2, label)
    if trace:
        return out, res
    return out


# revision 11
# speedup vs baseline: 2.9251x; 1.2088x over previous
"""Cluster-loss (two-view) Trainium2 kernel — label-sorted windowed segsum.

Math:
    f1n = feat1 / ||feat1||_row ;  f2n = feat2 / ||feat2||_row
    hseg = segsum(f1n - f2n, label)  (= s1 - s2)
    loss = sum_over_present_classes relu(||hseg_c / max(cnt_c,1)||^2 - margin)

Device strategy (per core):
  Host sorts all tokens by label and assigns each core a contiguous range of
  whole classes with ~N/8 tokens (classes are disjoint across cores -> no
  all-reduce; per-core partial IS the final segment sum for its classes).
  Because tokens arrive class-sorted, each 2048-token batch only touches a
  ~3-class span whose position follows a fixed schedule (1000 tokens/class);
  the one-hot matmul thus needs only a WIN=24-column window instead of 1024:
      psum[d, BASE(b):BASE(b)+24] += f_tile[tok,d]^T @ W[tok,24]
  W[tok, j] = (iota[j] == lab_rel[tok]) * rs[tok],  lab_rel = local_class -
  BASE(b) (host-computed; tokens that would fall outside the window - none in
  practice - are routed to an exact host fp64 fallback path instead).
  feat2 is negated on the host so one PSUM region accumulates s1 - s2.

Per-batch device pipeline (five stages, software-pipelined):
  S0 DMA:  SWDGE loads fp32->fp16 cast in flight (8 KiB contig / partition)
  S1 ACT:  sq = Square(f)  [128, 2048] batched
  S2 DVE:  4 halving adds (fp16 2x mode) + tensor_reduce -> ss[tok]
  S3 ACT:  rs = 1/sqrt(ss)  (Abs_reciprocal_sqrt LUT)
  S4 DVE+PE: per 128-token tile: W build (tensor_scalar is_eq*mult) + matmul

Sharding: class-range partition of the globally label-sorted token order.
"""

from contextlib import ExitStack

import ml_dtypes
import numpy as np

FP8 = ml_dtypes.float8_e4m3

import concourse.bass as bass
import concourse.mybir as mybir
import concourse.tile as tile
from concourse import bacc
from concourse.bass_utils import run_bass_kernel_spmd

N_CORES = 8
D = 128
C = 1000
P = 128                  # tokens per tile (matmul K)
TPB = 16                 # tiles per batch
TOK_B = P * TPB          # 2048 tokens per batch
NB = 62                  # batches per core
NT = NB * TPB            # 992 tiles per core
SHARD = NB * TOK_B       # 126976 token capacity per core
WIN = 24                 # one-hot window columns
GUARD = 8                # window guard below predicted class position
OUTW = 192               # per-core PSUM output width (>= max classes/core)
MARGIN = 0.1
TOK_PER_CLASS = 1000.0   # schedule rate: expected tokens per class

F32 = mybir.dt.float32
F16 = mybir.dt.float16
F8 = mybir.dt.float8e4
AF = mybir.ActivationFunctionType
OP = mybir.AluOpType


def win_base(b: int) -> int:
    """Fixed per-batch window base (local class index), even-aligned."""
    pred = int(b * TOK_B / TOK_PER_CLASS) - GUARD
    pred &= ~1
    return max(0, min(OUTW - WIN, pred))


def build_nc(nb: int = NB):
    nt = nb * TPB
    shard = nt * P
    nc = bacc.Bacc("TRN2", target_bir_lowering=False, debug=False)

    f1_d = nc.dram_tensor("f1", [shard, D], F8, kind="ExternalInput")
    f2_d = nc.dram_tensor("f2", [shard, D], F8, kind="ExternalInput")
    lab_d = nc.dram_tensor("lab", [P, nt], F32, kind="ExternalInput")
    iota_d = nc.dram_tensor("iota", [P, TPB * WIN], F16, kind="ExternalInput")
    out_d = nc.dram_tensor("hseg", [D, OUTW], F32, kind="ExternalOutput")

    # token s = b*2048 + p*16 + t  ->  partition p reads 16*128 contiguous fp32
    f1r = f1_d.ap().rearrange("(b p t) d -> b p (t d)", p=P, t=TPB)
    f2r = f2_d.ap().rearrange("(b p t) d -> b p (t d)", p=P, t=TPB)

    with tile.TileContext(nc) as tc, ExitStack() as ctx:
        const = ctx.enter_context(tc.tile_pool(name="const", bufs=1))
        fpool = ctx.enter_context(tc.tile_pool(name="fpool", bufs=7))
        sqpool = ctx.enter_context(tc.tile_pool(name="sqpool", bufs=3))
        trpool = ctx.enter_context(tc.tile_pool(name="trpool", bufs=3))
        spool = ctx.enter_context(tc.tile_pool(name="spool", bufs=nb))
        wpool = ctx.enter_context(tc.tile_pool(name="wpool", bufs=10))
        ppool = ctx.enter_context(tc.tile_pool(name="ppool", bufs=1, space="PSUM"))

        iota_sb = const.tile([P, TPB * WIN], F16)
        nc.sync.dma_start(iota_sb[:], iota_d[:])
        lab_sb = const.tile([P, nt], F32)
        nc.sync.dma_start(lab_sb[:], lab_d[:])
        zeros = const.tile([P, OUTW], F16)
        nc.vector.memset(zeros[:], 0.0)

        psum = ppool.tile([D, OUTW], F32)
        # zero the whole accumulation window: 0^T @ 0
        nc.tensor.matmul(psum[:], zeros[:, 0:P], zeros[:], start=True, stop=False)

        def emit_load(b, st):
            f1t = fpool.tile([P, TPB * D], F16, name="f1t")
            nc.gpsimd.dma_start(f1t[:], f1r[b])
            f2t = fpool.tile([P, TPB * D], F16, name="f2t")
            nc.gpsimd.dma_start(f2t[:], f2r[b])
            st["f1t"], st["f2t"] = f1t, f2t

        def emit_square(st):
            for v in (1, 2):
                sq = sqpool.tile([P, TPB * D], F16, name=f"sq{v}")
                nc.scalar.activation(sq[:], st[f"f{v}t"][:], AF.Square)
                st[f"sq{v}"] = sq

        def emit_tree(st):
            for v in (1, 2):
                src = st[f"sq{v}"][:].rearrange("p (t d) -> p t d", d=D)
                w = D
                while w > 8:
                    w //= 2
                    dst_t = trpool.tile([P, TPB * w], F16, name=f"l{w}_{v}")
                    dst = dst_t[:].rearrange("p (t d) -> p t d", d=w)
                    nc.vector.tensor_tensor(
                        dst, src[:, :, 0:w], src[:, :, w : 2 * w], OP.add
                    )
                    src = dst
                ss = spool.tile([P, TPB], F32, name=f"ss{v}")
                nc.vector.tensor_reduce(
                    ss[:], src, axis=mybir.AxisListType.X, op=OP.add
                )
                st[f"ss{v}"] = ss

        def emit_rs(st):
            for v in (1, 2):
                rs = spool.tile([P, TPB], F32, name=f"rs{v}")
                nc.scalar.activation(rs[:], st[f"ss{v}"][:], AF.Abs_reciprocal_sqrt)
                st[f"rs{v}"] = rs

        def emit_w(st):
            """Whole-batch one-hot build: 3 DVE ops instead of 32.
            eq[p, t, j] = (iota[j] == lab_rel[p, t]);  w_v = eq * rs_v[p, t]."""
            b = st["b"]
            sl = slice(b * TPB, (b + 1) * TPB)
            eq = wpool.tile([P, TPB * WIN], F16, name="eq")
            lab_bc = lab_sb[:, sl].unsqueeze(-1).broadcast_to([P, TPB, WIN])
            nc.vector.tensor_tensor(
                eq[:].rearrange("p (t j) -> p t j", j=WIN),
                iota_sb[:].rearrange("p (t j) -> p t j", j=WIN),
                lab_bc,
                OP.is_equal,
            )
            for v in (1, 2):
                w = wpool.tile([P, TPB * WIN], F16, name=f"w{v}")
                rs_bc = st[f"rs{v}"][:].unsqueeze(-1).broadcast_to([P, TPB, WIN])
                nc.vector.tensor_tensor(
                    w[:].rearrange("p (t j) -> p t j", j=WIN),
                    eq[:].rearrange("p (t j) -> p t j", j=WIN),
                    rs_bc,
                    OP.mult,
                )
                st[f"w{v}"] = w

        def emit_mm(st):
            b = st["b"]
            base = win_base(b)
            last_b = b == nb - 1
            for t in range(TPB):
                for v in (1, 2):
                    stop = last_b and t == TPB - 1 and v == 2
                    nc.tensor.matmul(
                        psum[:, base : base + WIN],
                        st[f"f{v}t"][:, t * D : (t + 1) * D],
                        st[f"w{v}"][:, t * WIN : (t + 1) * WIN],
                        start=False, stop=stop,
                    )

        # five-stage software pipeline over batches
        sts = {}
        for k in range(nb + 5):
            if k < nb:
                sts[k] = {"b": k}
                emit_load(k, sts[k])
            if 0 <= k - 2 < nb:
                emit_square(sts[k - 2])
            if 0 <= k - 3 < nb:
                emit_tree(sts[k - 3])
            if 0 <= k - 4 < nb:
                emit_rs(sts[k - 4])
            if 0 <= k - 5 < nb:
                emit_w(sts[k - 5])
                emit_mm(sts[k - 5])
                del sts[k - 5]

        outsb = const.tile([D, OUTW], F32)
        nc.scalar.copy(outsb[:], psum[:])
        nc.sync.dma_start(out_d[:], outsb[:])

    nc.compile()
    return nc


_NC_CACHE = {}


def _get_nc(nb: int = NB):
    if nb not in _NC_CACHE:
        _NC_CACHE[nb] = build_nc(nb)
    return _NC_CACHE[nb]


def _plan_shards(label: np.ndarray):
    """Class-range partition of the sorted tokens: 8 ranges of whole classes,
    each with <= SHARD tokens. Returns (order, class_bounds, counts)."""
    order = np.argsort(label, kind="stable")
    counts = np.bincount(label, minlength=C)
    cum = np.concatenate([[0], np.cumsum(counts)])  # [C+1]
    n = label.shape[0]
    bounds = [0]
    for i in range(1, N_CORES):
        target = round(i * n / N_CORES)
        c = int(np.searchsorted(cum, target))
        if c > 0 and abs(cum[c - 1] - target) < abs(cum[c] - target):
            c -= 1
        c = max(c, bounds[-1])
        bounds.append(min(c, C))
    bounds.append(C)
    # enforce per-range capacity
    for _ in range(C):
        ok = True
        for i in range(N_CORES):
            if cum[bounds[i + 1]] - cum[bounds[i]] > SHARD:
                if i + 1 < N_CORES:
                    bounds[i + 1] -= 1
                else:
                    bounds[i] += 1  # steal from previous range
                ok = False
        if ok:
            break
    for i in range(N_CORES):
        cnt = cum[bounds[i + 1]] - cum[bounds[i]]
        assert cnt <= SHARD, f"shard {i} over capacity: {cnt}"
        assert bounds[i + 1] - bounds[i] <= OUTW, "class range too wide"
    return order, bounds, counts, cum


def make_in_maps(feat1, feat2, label):
    order, bounds, counts, cum = _plan_shards(label)
    sorted_labels = label[order]
    iota = np.ascontiguousarray(
        np.broadcast_to(
            np.tile(np.arange(WIN, dtype=np.float16), TPB), (P, TPB * WIN)
        )
    )
    base_of_batch = np.array([win_base(b) for b in range(NB)], dtype=np.int64)
    base_per_tok = np.repeat(base_of_batch, TOK_B)  # [SHARD]

    in_maps, spill_idx = [], []
    for i in range(N_CORES):
        s0, s1 = int(cum[bounds[i]]), int(cum[bounds[i + 1]])
        cnt = s1 - s0
        sel = order[s0:s1]
        f1c = np.ones((SHARD, D), dtype=FP8)
        f1c[:cnt] = feat1[sel].astype(FP8)
        f2c = np.ones((SHARD, D), dtype=FP8)
        f2c[:cnt] = (-feat2[sel]).astype(FP8)
        local = sorted_labels[s0:s1] - bounds[i]
        lab_rel = local - base_per_tok[:cnt]
        bad = (lab_rel < 0) | (lab_rel >= WIN)
        if bad.any():
            spill_idx.append(sel[bad])
            lab_rel = lab_rel.copy()
            lab_rel[bad] = -1000
        lab_full = np.full(SHARD, -1000.0, dtype=np.float32)
        lab_full[:cnt] = lab_rel.astype(np.float32)
        lab_dev = np.ascontiguousarray(
            lab_full.reshape(NB, P, TPB).transpose(1, 0, 2).reshape(P, NT)
        )
        in_maps.append({"f1": f1c, "f2": f2c, "lab": lab_dev, "iota": iota})
    spill = np.concatenate(spill_idx) if spill_idx else np.empty(0, dtype=np.int64)
    return in_maps, bounds, counts, spill


def finish_host(outs, bounds, counts, spill, feat1, feat2, label):
    hseg = np.zeros((D, C), dtype=np.float64)
    for i in range(N_CORES):
        k = bounds[i + 1] - bounds[i]
        hseg[:, bounds[i] : bounds[i + 1]] += outs[i][:, :k].astype(np.float64)
    if spill.size:
        r1 = feat1[spill].astype(np.float64)
        r2 = feat2[spill].astype(np.float64)
        h = r1 / np.sqrt((r1 * r1).sum(1, keepdims=True)) - r2 / np.sqrt(
            (r2 * r2).sum(1, keepdims=True)
        )
        np.add.at(hseg.T, label[spill], h)
    denom = np.maximum(counts, 1.0)
    cdiff = hseg / denom[None, :]
    per_class = (cdiff * cdiff).sum(0)
    hinge = np.maximum(per_class - MARGIN, 0.0)
    hinge = np.where(counts > 0, hinge, 0.0)
    return np.array(hinge.sum(), dtype=np.float32)


def kernel(feat1, feat2, label1, trace: bool = False):
    feat1 = np.ascontiguousarray(np.asarray(feat1, dtype=np.float32))
    feat2 = np.ascontiguousarray(np.asarray(feat2, dtype=np.float32))
    label = np.asarray(label1).astype(np.int64)

    in_maps, bounds, counts, spill = make_in_maps(feat1, feat2, label)
    nc = _get_nc()
    res = run_bass_kernel_spmd(
        nc, in_maps, core_ids=list(range(N_CORES)), trace=trace
    )
    outs = [res.results[i]["hseg"] for i in range(N_CORES)]
    out = finish_host(outs, bounds, counts, spill, feat1, feat2, label)
    if trace:
        return out, res
    return out
